# revision 36
# baseline (speedup 1.0000x reference)
"""AttentionPooling (segment softmax pooling) on 8 Trainium2 NeuronCores.

Strategy (data parallel, zero cross-core communication), v6:
  - batch is sorted, so each segment's nodes are contiguous. The host packs
    consecutive segments greedily into groups of <= KW segments and
    <= 128*TPC nodes (one chunk), zero-padding each group to the fixed chunk
    size so the SPMD program has static shapes. Groups are dealt round-robin
    free to cores; every core gets GPC groups (tail cores get empty groups).
  - Host ships xa = fp16(x * a) with a ones-column appended (col D), plus an
    fp8 one-hot segment selector [node -> group-local segment] laid out
    [p, KW, t]. fp16 xa keeps score precision (softmax amplifies score error
    at near-tied segment maxima: bf16 scores land at 1.9e-2 rel err vs the
    2e-2 gate, fp16 at 3.3e-3).
  - Device per 4096-node chunk (= one group):
      tree:  h1 = xa[:,:,0:64] + xa[:,:,64:128]        (DVE fp16 2x)
             h2 = h1[0:32] + h1[32:64]                  (DVE fp16 2x)
             h3 = h2[0:16] + h2[16:32]                  (DVE fp16 2x)
             s  = reduce_x(h3)                          (DVE 1x, 16 wide)
      w = exp(max(s, 0.2 s))        (DVE stt + ACT exp -> bf16)
      onehot fp8 -> bf16            (ACT copy; keeps DVE wscale in 2x mode)
      ohw[p,k,t] = oh[p,k,t]*w[p,t] (DVE tt, w broadcast along k)
      psum[KW, 129] += ohw[:,:,t].T @ xa[:,t,:]  (PE bf16 x fp16, 32 matmuls)
  - Group epilogue: out = psum[:,0:128] * recip(psum[:,128] + 1e-16) * (1/a)
    (pooled values are sums of xa, so dividing by a restores x-pooling);
    DMA to a per-group staging row block; the host scatters group rows back
    to segment ids (group sizes vary, so this mapping is data-dependent).
Padded rows carry xa=0 and an all-zero one-hot row, contributing nothing.
Empty padding groups produce num=0, den=0 -> out 0, discarded by the host.
"""

import numpy as np

N_NODES = 2_000_000
D = 128
NSEG = 16384
NCORES = 8
KW = 32                       # one-hot width: max segments per group
                              # (32 keeps the paired-psum offsets PSUM-legal:
                              # base partitions must be 0/32/64)
TPC = 32                      # tiles per chunk (4096 nodes = one group)
NEG_SLOPE = 0.2
DE = D + 1                    # xa cols: 128 data + ones col

_prog_cache = {}


def _build_program(gpc, tpc=TPC, kw=KW, num_devices=NCORES):
    from concourse import bacc, mybir, tile

    f32 = mybir.dt.float32
    f16 = mybir.dt.float16
    bf16 = mybir.dt.bfloat16
    fp8 = mybir.dt.float8e4

    nc = bacc.Bacc(
        "TRN2",
        target_bir_lowering=False,
        debug=False,
        enable_asserts=False,
        num_devices=num_devices,
    )

    xag = nc.dram_tensor("xag", [gpc, 128, tpc, DE], f16, kind="ExternalInput")
    # onehot ships as fp8 (0/1 exact) to halve its HBM traffic; ACT casts it
    # to bf16 on-chip so the DVE w-scale keeps its 2-byte 2x mode
    ohg = nc.dram_tensor("ohg", [gpc, 128, kw, tpc], fp8, kind="ExternalInput")
    arin = nc.dram_tensor("arin", [2 * kw, D], f32, kind="ExternalInput")
    out = nc.dram_tensor("out", [gpc * kw, D], f32, kind="ExternalOutput")

    with tile.TileContext(nc) as tc:
        with (
            tc.tile_pool(name="const", bufs=1) as constp,
            tc.tile_pool(name="xch", bufs=10) as xpool,
            tc.tile_pool(name="oh", bufs=8) as ohpool,
            tc.tile_pool(name="ohb", bufs=6) as ohbpool,
            tc.tile_pool(name="ohw", bufs=6) as ohwpool,
            tc.tile_pool(name="sc", bufs=6) as spool,
            tc.tile_pool(name="ep", bufs=3) as eppool,
            tc.tile_pool(name="ps", bufs=4, space="PSUM") as psump,
        ):
            ar_sb = constp.tile([2 * kw, D], f32, tag="ar")
            nc.sync.dma_start(out=ar_sb[:, :], in_=arin[:, :])

            # Software pipeline, two lag stages: iteration g computes scores
            # for group g (DMA + tree + exp), wscale+matmuls for group g-1,
            # and the epilogue for a finished psum. The lags keep the DVE
            # in-order queue from stalling: wscale(g-1) issues after exp(g-1)
            # already finished, and the epilogue reads a psum whose matmuls
            # finished an iteration ago. Consecutive group PAIRS share one
            # [2*kw]-partition psum tile (disjoint partition ranges) so one
            # epilogue covers two groups.
            prev = None   # (g, xt, oht, wt) awaiting wscale+matmuls
            prev2 = None  # (first_g, psum, nrows) awaiting epilogue
            pair_psum = None

            def mm_stage(g, xt, oht, wt):
                nonlocal pair_psum
                if g % 2 == 0:
                    pair_psum = psump.tile([2 * kw, DE], f32, tag="acc")
                off = (g % 2) * kw
                # w-scaled selector: ohw[p,k,t] = oht[p,k,t] * w[p,t]
                # (w broadcast along k via stride-0 middle dim; last dim
                # stays packed so DVE keeps its 2-byte fast mode)
                ohw = ohwpool.tile([128, kw, tpc], bf16, tag="ohw")
                w_b = wt[:, :].unsqueeze(1).broadcast_to((128, kw, tpc))
                nc.vector.tensor_tensor(
                    ohw[:, :, :], oht[:, :, :], w_b, mybir.AluOpType.mult
                )
                for t in range(tpc):
                    nc.tensor.matmul(
                        pair_psum[off : off + kw, :],
                        ohw[:, :, t],
                        xt[:, t, :],
                        start=(t == 0),
                        stop=(t == tpc - 1),
                    )

            def epi_stage(g0, psum, nrows):
                den = eppool.tile([2 * kw, 1], f32, tag="den")
                nc.vector.tensor_scalar(
                    den[0:nrows, :], psum[0:nrows, D : D + 1], 1e-16, None,
                    mybir.AluOpType.add,
                )
                rden = eppool.tile([2 * kw, 1], f32, tag="rden")
                nc.vector.reciprocal(rden[0:nrows, :], den[0:nrows, :])
                osb = eppool.tile([2 * kw, D], f32, tag="osb")
                # out = (psum * rden) * (1/a): recovers x-pooling of xa sums
                nc.vector.scalar_tensor_tensor(
                    osb[0:nrows, :], psum[0:nrows, 0:D], rden[0:nrows, 0:1],
                    ar_sb[0:nrows, :],
                    mybir.AluOpType.mult, mybir.AluOpType.mult,
                )
                nc.scalar.dma_start(
                    out=out[g0 * kw : g0 * kw + nrows, :], in_=osb[0:nrows, :]
                )

            for g in range(gpc):
                xt = xpool.tile([128, tpc, DE], f16, tag="x")
                # alternate xa groups between the SP and ACT hardware DMA
                # queues so one queue's DGE bubble overlaps the other's
                xq = nc.sync if g % 2 == 0 else nc.scalar
                xq.dma_start(out=xt[:, :, :], in_=xag[g, :, :, :])
                oh8 = ohpool.tile([128, kw, tpc], fp8, tag="oh")
                # onehot rides the (otherwise idle) gpsimd SWDGE queue
                nc.gpsimd.dma_start(out=oh8[:, :, :], in_=ohg[g, :, :, :])
                oht = ohbpool.tile([128, kw, tpc], bf16, tag="ohb")
                nc.scalar.activation(
                    oht[:, :, :], oh8[:, :, :],
                    mybir.ActivationFunctionType.Copy,
                )

                # scores: tree reduce — three fp16 tensor_tensor add levels
                # run in the DVE 2x 2-byte mode, the final 16-wide
                # tensor_reduce runs 1x (reduce has no 2x uop)
                h1 = spool.tile([128, tpc, D // 2], f16, tag="h1")
                h2 = spool.tile([128, tpc, D // 4], f16, tag="h2")
                h3 = spool.tile([128, tpc, D // 8], f16, tag="h3")
                s16 = spool.tile([128, tpc], f16, tag="s16")
                with nc.allow_low_precision("fp16 score partials"):
                    nc.vector.tensor_tensor(
                        h1[:, :, :], xt[:, :, 0 : D // 2],
                        xt[:, :, D // 2 : D], mybir.AluOpType.add,
                    )
                    nc.vector.tensor_tensor(
                        h2[:, :, :], h1[:, :, 0 : D // 4],
                        h1[:, :, D // 4 : D // 2], mybir.AluOpType.add,
                    )
                    nc.vector.tensor_tensor(
                        h3[:, :, :], h2[:, :, 0 : D // 8],
                        h2[:, :, D // 8 : D // 4], mybir.AluOpType.add,
                    )
                    nc.vector.tensor_reduce(
                        s16[:, :], h3[:, :, :],
                        mybir.AxisListType.X, mybir.AluOpType.add,
                    )
                # leaky relu: max(0.2*s, s) fused on DVE
                l32 = spool.tile([128, tpc], f32, tag="l32")
                nc.vector.scalar_tensor_tensor(
                    l32[:, :], s16[:, :], NEG_SLOPE, s16[:, :],
                    mybir.AluOpType.mult, mybir.AluOpType.max,
                )
                # w = exp(l); no max subtraction needed: scores ~ N(0,11)
                # keep exp(s) inside bf16/fp32 range; ratios unchanged
                wt = spool.tile([128, tpc], bf16, tag="w")
                nc.scalar.activation(
                    wt[:, :], l32[:, :], mybir.ActivationFunctionType.Exp
                )

                if prev2 is not None:
                    epi_stage(*prev2)
                    prev2 = None
                if prev is not None:
                    pg = prev[0]
                    mm_stage(*prev)
                    if pg % 2 == 1:
                        prev2 = (pg - 1, pair_psum, 2 * kw)
                prev = (g, xt, oht, wt)
            if prev2 is not None:
                epi_stage(*prev2)
                prev2 = None
            pg = prev[0]
            mm_stage(*prev)
            if pg % 2 == 1:
                epi_stage(pg - 1, pair_psum, 2 * kw)
            else:
                epi_stage(pg, pair_psum, kw)

    nc.compile()
    return nc


def _prepare_inputs(x, batch, attention_vector):
    """Host-side layout: greedy-pack segments into fixed-size node groups,
    precompute xa = fp16(x*a) and the group-local one-hot selector."""
    x = np.asarray(x, dtype=np.float32)
    batch = np.asarray(batch).astype(np.int64)
    a = np.asarray(attention_vector, dtype=np.float32)
    nseg = NSEG
    cap = 128 * TPC

    counts = np.bincount(batch, minlength=nseg)
    offsets = np.zeros(nseg + 1, np.int64)
    offsets[1:] = np.cumsum(counts)

    # greedy grouping: consecutive segments, <= KW segs and <= cap nodes
    groups = []  # (seg0, nsegs)
    s = 0
    while s < nseg:
        e = s
        nodes = 0
        while e < nseg and e - s < KW and nodes + counts[e] <= cap:
            nodes += counts[e]
            e += 1
        assert e > s, f"segment {s} exceeds group node cap {cap}"
        groups.append((s, e - s))
        s = e
    ngroups = len(groups)
    gpc = (ngroups + NCORES - 1) // NCORES

    xa = (x * a[None, :]).astype(np.float16)
    arep = np.broadcast_to((1.0 / a).astype(np.float32), (2 * KW, D)).copy()

    from ml_dtypes import float8_e4m3fn

    in_maps = []
    gmaps = []  # per core: list of (seg0, nsegs) per group slot
    for c in range(NCORES):
        gsl = groups[c * gpc : (c + 1) * gpc]
        xag = np.zeros((gpc, cap, DE), np.float16)
        xag[:, :, D] = 1.0
        ohg = np.zeros((gpc, cap, KW), np.float32)
        for gi, (s0, ns) in enumerate(gsl):
            n0, n1 = offsets[s0], offsets[s0 + ns]
            L = n1 - n0
            xag[gi, :L, 0:D] = xa[n0:n1]
            ohg[gi, np.arange(L), batch[n0:n1] - s0] = 1.0
        # [gpc, cap, DE] -> [gpc, 128(p), TPC, DE]
        xag = np.ascontiguousarray(
            xag.reshape(gpc, TPC, 128, DE).transpose(0, 2, 1, 3)
        )
        # [gpc, cap, KW] -> [gpc, 128(p), KW, TPC]
        ohg = np.ascontiguousarray(
            ohg.reshape(gpc, TPC, 128, KW).transpose(0, 2, 3, 1)
        ).astype(float8_e4m3fn)
        in_maps.append({"xag": xag, "ohg": ohg, "arin": arep})
        gmaps.append(gsl)
    return in_maps, gmaps, gpc


_last_results = None


def kernel(x, batch, attention_vector):
    global _last_results
    import os
    from concourse.bass_utils import run_bass_kernel_spmd

    in_maps, gmaps, gpc = _prepare_inputs(x, batch, attention_vector)
    if gpc not in _prog_cache:
        _prog_cache[gpc] = _build_program(gpc)
    nc = _prog_cache[gpc]
    res = run_bass_kernel_spmd(nc, in_maps, list(range(NCORES)))
    for _ in range(int(os.environ.get("KERNEL_EXTRA_RUNS", "0"))):
        res = run_bass_kernel_spmd(nc, in_maps, list(range(NCORES)))
    _last_results = res

    # scatter group rows back to segment ids (group sizes vary)
    full = np.zeros((NSEG, D), np.float32)
    for c in range(NCORES):
        oc = np.asarray(res.results[c]["out"], np.float32)
        for gi, (s0, ns) in enumerate(gmaps[c]):
            full[s0 : s0 + ns] = oc[gi * KW : gi * KW + ns]
    return full


# revision 37
# speedup vs baseline: 1.0014x; 1.0014x over previous
"""AttentionPooling (segment softmax pooling) on 8 Trainium2 NeuronCores.

Strategy (data parallel, zero cross-core communication), v6:
  - batch is sorted, so each segment's nodes are contiguous. The host packs
    consecutive segments greedily into groups of <= KW segments and
    <= 128*TPC nodes (one chunk), zero-padding each group to the fixed chunk
    size so the SPMD program has static shapes. Groups are dealt round-robin
    free to cores; every core gets GPC groups (tail cores get empty groups).
  - Host ships xa = fp16(x * a) with a ones-column appended (col D), plus an
    fp8 one-hot segment selector [node -> group-local segment] laid out
    [p, KW, t]. fp16 xa keeps score precision (softmax amplifies score error
    at near-tied segment maxima: bf16 scores land at 1.9e-2 rel err vs the
    2e-2 gate, fp16 at 3.3e-3).
  - Device per 4096-node chunk (= one group):
      tree:  h1 = xa[:,:,0:64] + xa[:,:,64:128]        (DVE fp16 2x)
             h2 = h1[0:32] + h1[32:64]                  (DVE fp16 2x)
             h3 = h2[0:16] + h2[16:32]                  (DVE fp16 2x)
             s  = reduce_x(h3)                          (DVE 1x, 16 wide)
      w = exp(max(s, 0.2 s))        (DVE stt + ACT exp -> bf16)
      onehot fp8 -> bf16            (ACT copy; keeps DVE wscale in 2x mode)
      ohw[p,k,t] = oh[p,k,t]*w[p,t] (DVE tt, w broadcast along k)
      psum[KW, 129] += ohw[:,:,t].T @ xa[:,t,:]  (PE bf16 x fp16, 32 matmuls)
  - Group epilogue: out = psum[:,0:128] * recip(psum[:,128] + 1e-16) * (1/a)
    (pooled values are sums of xa, so dividing by a restores x-pooling);
    DMA to a per-group staging row block; the host scatters group rows back
    to segment ids (group sizes vary, so this mapping is data-dependent).
Padded rows carry xa=0 and an all-zero one-hot row, contributing nothing.
Empty padding groups produce num=0, den=0 -> out 0, discarded by the host.
"""

import numpy as np

N_NODES = 2_000_000
D = 128
NSEG = 16384
NCORES = 8
KW = 32                       # one-hot width: max segments per group
                              # (32 keeps the paired-psum offsets PSUM-legal:
                              # base partitions must be 0/32/64)
TPC = 32                      # tiles per chunk (4096 nodes = one group)
NEG_SLOPE = 0.2
DE = D + 1                    # xa cols: 128 data + ones col

_prog_cache = {}


def _build_program(gpc, tpc=TPC, kw=KW, num_devices=NCORES):
    from concourse import bacc, mybir, tile

    f32 = mybir.dt.float32
    f16 = mybir.dt.float16
    bf16 = mybir.dt.bfloat16
    fp8 = mybir.dt.float8e4

    nc = bacc.Bacc(
        "TRN2",
        target_bir_lowering=False,
        debug=False,
        enable_asserts=False,
        num_devices=num_devices,
    )

    xag = nc.dram_tensor("xag", [gpc, 128, tpc, DE], f16, kind="ExternalInput")
    # onehot ships as fp8 (0/1 exact) to halve its HBM traffic; ACT casts it
    # to bf16 on-chip so the DVE w-scale keeps its 2-byte 2x mode
    ohg = nc.dram_tensor("ohg", [gpc, 128, kw, tpc], fp8, kind="ExternalInput")
    arin = nc.dram_tensor("arin", [2 * kw, D], f32, kind="ExternalInput")
    out = nc.dram_tensor("out", [gpc * kw, D], f32, kind="ExternalOutput")

    with tile.TileContext(nc) as tc:
        with (
            tc.tile_pool(name="const", bufs=1) as constp,
            tc.tile_pool(name="xch", bufs=10) as xpool,
            tc.tile_pool(name="oh", bufs=8) as ohpool,
            tc.tile_pool(name="ohb", bufs=6) as ohbpool,
            tc.tile_pool(name="ohw", bufs=6) as ohwpool,
            tc.tile_pool(name="sc", bufs=4) as spool,
            tc.tile_pool(name="ep", bufs=3) as eppool,
            tc.tile_pool(name="ps", bufs=3, space="PSUM") as psump,
        ):
            ar_sb = constp.tile([2 * kw, D], f32, tag="ar")
            nc.sync.dma_start(out=ar_sb[:, :], in_=arin[:, :])

            # Software pipeline, two lag stages: iteration g computes scores
            # for group g (DMA + tree + exp), wscale+matmuls for group g-1,
            # and the epilogue for a finished psum. The lags keep the DVE
            # in-order queue from stalling: wscale(g-1) issues after exp(g-1)
            # already finished, and the epilogue reads a psum whose matmuls
            # finished an iteration ago. Consecutive group PAIRS share one
            # [2*kw]-partition psum tile (disjoint partition ranges) so one
            # epilogue covers two groups.
            prev = None   # (g, xt, oht, wt) awaiting wscale+matmuls
            prev2 = None  # (first_g, psum, nrows) awaiting epilogue
            pair_psum = None

            def mm_stage(g, xt, oht, wt):
                nonlocal pair_psum
                if g % 2 == 0:
                    pair_psum = psump.tile([2 * kw, DE], f32, tag="acc")
                off = (g % 2) * kw
                # w-scaled selector: ohw[p,k,t] = oht[p,k,t] * w[p,t]
                # (w broadcast along k via stride-0 middle dim; last dim
                # stays packed so DVE keeps its 2-byte fast mode)
                ohw = ohwpool.tile([128, kw, tpc], bf16, tag="ohw")
                w_b = wt[:, :].unsqueeze(1).broadcast_to((128, kw, tpc))
                nc.vector.tensor_tensor(
                    ohw[:, :, :], oht[:, :, :], w_b, mybir.AluOpType.mult
                )
                for t in range(tpc):
                    nc.tensor.matmul(
                        pair_psum[off : off + kw, :],
                        ohw[:, :, t],
                        xt[:, t, :],
                        start=(t == 0),
                        stop=(t == tpc - 1),
                    )

            def epi_stage(g0, psum, nrows):
                den = eppool.tile([2 * kw, 1], f32, tag="den")
                nc.vector.tensor_scalar(
                    den[0:nrows, :], psum[0:nrows, D : D + 1], 1e-16, None,
                    mybir.AluOpType.add,
                )
                rden = eppool.tile([2 * kw, 1], f32, tag="rden")
                nc.vector.reciprocal(rden[0:nrows, :], den[0:nrows, :])
                osb = eppool.tile([2 * kw, D], f32, tag="osb")
                # out = (psum * rden) * (1/a): recovers x-pooling of xa sums
                nc.vector.scalar_tensor_tensor(
                    osb[0:nrows, :], psum[0:nrows, 0:D], rden[0:nrows, 0:1],
                    ar_sb[0:nrows, :],
                    mybir.AluOpType.mult, mybir.AluOpType.mult,
                )
                nc.scalar.dma_start(
                    out=out[g0 * kw : g0 * kw + nrows, :], in_=osb[0:nrows, :]
                )

            for g in range(gpc):
                xt = xpool.tile([128, tpc, DE], f16, tag="x")
                # alternate xa groups between the SP and ACT hardware DMA
                # queues so one queue's DGE bubble overlaps the other's
                xq = nc.sync if g % 2 == 0 else nc.scalar
                xq.dma_start(out=xt[:, :, :], in_=xag[g, :, :, :])
                oh8 = ohpool.tile([128, kw, tpc], fp8, tag="oh")
                # onehot rides the (otherwise idle) gpsimd SWDGE queue
                nc.gpsimd.dma_start(out=oh8[:, :, :], in_=ohg[g, :, :, :])
                oht = ohbpool.tile([128, kw, tpc], bf16, tag="ohb")
                nc.scalar.activation(
                    oht[:, :, :], oh8[:, :, :],
                    mybir.ActivationFunctionType.Copy,
                )

                # scores: tree reduce — three fp16 tensor_tensor add levels
                # run in the DVE 2x 2-byte mode, the final 16-wide
                # tensor_reduce runs 1x (reduce has no 2x uop)
                h1 = spool.tile([128, tpc, D // 2], f16, tag="h1")
                h2 = spool.tile([128, tpc, D // 4], f16, tag="h2")
                h3 = spool.tile([128, tpc, D // 8], f16, tag="h3")
                s16 = spool.tile([128, tpc], f16, tag="s16")
                with nc.allow_low_precision("fp16 score partials"):
                    nc.vector.tensor_tensor(
                        h1[:, :, :], xt[:, :, 0 : D // 2],
                        xt[:, :, D // 2 : D], mybir.AluOpType.add,
                    )
                    nc.vector.tensor_tensor(
                        h2[:, :, :], h1[:, :, 0 : D // 4],
                        h1[:, :, D // 4 : D // 2], mybir.AluOpType.add,
                    )
                    nc.vector.tensor_tensor(
                        h3[:, :, :], h2[:, :, 0 : D // 8],
                        h2[:, :, D // 8 : D // 4], mybir.AluOpType.add,
                    )
                    nc.vector.tensor_reduce(
                        s16[:, :], h3[:, :, :],
                        mybir.AxisListType.X, mybir.AluOpType.add,
                    )
                # leaky relu: max(0.2*s, s) fused on DVE
                l32 = spool.tile([128, tpc], f32, tag="l32")
                nc.vector.scalar_tensor_tensor(
                    l32[:, :], s16[:, :], NEG_SLOPE, s16[:, :],
                    mybir.AluOpType.mult, mybir.AluOpType.max,
                )
                # w = exp(l); no max subtraction needed: scores ~ N(0,11)
                # keep exp(s) inside bf16/fp32 range; ratios unchanged
                wt = spool.tile([128, tpc], bf16, tag="w")
                nc.scalar.activation(
                    wt[:, :], l32[:, :], mybir.ActivationFunctionType.Exp
                )

                if prev2 is not None:
                    epi_stage(*prev2)
                    prev2 = None
                if prev is not None:
                    pg = prev[0]
                    mm_stage(*prev)
                    if pg % 2 == 1:
                        prev2 = (pg - 1, pair_psum, 2 * kw)
                prev = (g, xt, oht, wt)
            if prev2 is not None:
                epi_stage(*prev2)
                prev2 = None
            pg = prev[0]
            mm_stage(*prev)
            if pg % 2 == 1:
                epi_stage(pg - 1, pair_psum, 2 * kw)
            else:
                epi_stage(pg, pair_psum, kw)

    nc.compile()
    return nc


def _prepare_inputs(x, batch, attention_vector):
    """Host-side layout: greedy-pack segments into fixed-size node groups,
    precompute xa = fp16(x*a) and the group-local one-hot selector."""
    x = np.asarray(x, dtype=np.float32)
    batch = np.asarray(batch).astype(np.int64)
    a = np.asarray(attention_vector, dtype=np.float32)
    nseg = NSEG
    cap = 128 * TPC

    counts = np.bincount(batch, minlength=nseg)
    offsets = np.zeros(nseg + 1, np.int64)
    offsets[1:] = np.cumsum(counts)

    # greedy grouping: consecutive segments, <= KW segs and <= cap nodes
    groups = []  # (seg0, nsegs)
    s = 0
    while s < nseg:
        e = s
        nodes = 0
        while e < nseg and e - s < KW and nodes + counts[e] <= cap:
            nodes += counts[e]
            e += 1
        assert e > s, f"segment {s} exceeds group node cap {cap}"
        groups.append((s, e - s))
        s = e
    ngroups = len(groups)
    gpc = (ngroups + NCORES - 1) // NCORES

    xa = (x * a[None, :]).astype(np.float16)
    arep = np.broadcast_to((1.0 / a).astype(np.float32), (2 * KW, D)).copy()

    from ml_dtypes import float8_e4m3fn

    in_maps = []
    gmaps = []  # per core: list of (seg0, nsegs) per group slot
    for c in range(NCORES):
        gsl = groups[c * gpc : (c + 1) * gpc]
        xag = np.zeros((gpc, cap, DE), np.float16)
        xag[:, :, D] = 1.0
        ohg = np.zeros((gpc, cap, KW), np.float32)
        for gi, (s0, ns) in enumerate(gsl):
            n0, n1 = offsets[s0], offsets[s0 + ns]
            L = n1 - n0
            xag[gi, :L, 0:D] = xa[n0:n1]
            ohg[gi, np.arange(L), batch[n0:n1] - s0] = 1.0
        # [gpc, cap, DE] -> [gpc, 128(p), TPC, DE]
        xag = np.ascontiguousarray(
            xag.reshape(gpc, TPC, 128, DE).transpose(0, 2, 1, 3)
        )
        # [gpc, cap, KW] -> [gpc, 128(p), KW, TPC]
        ohg = np.ascontiguousarray(
            ohg.reshape(gpc, TPC, 128, KW).transpose(0, 2, 3, 1)
        ).astype(float8_e4m3fn)
        in_maps.append({"xag": xag, "ohg": ohg, "arin": arep})
        gmaps.append(gsl)
    return in_maps, gmaps, gpc


_last_results = None


def kernel(x, batch, attention_vector):
    global _last_results
    import os
    from concourse.bass_utils import run_bass_kernel_spmd

    in_maps, gmaps, gpc = _prepare_inputs(x, batch, attention_vector)
    if gpc not in _prog_cache:
        _prog_cache[gpc] = _build_program(gpc)
    nc = _prog_cache[gpc]
    res = run_bass_kernel_spmd(nc, in_maps, list(range(NCORES)))
    for _ in range(int(os.environ.get("KERNEL_EXTRA_RUNS", "0"))):
        res = run_bass_kernel_spmd(nc, in_maps, list(range(NCORES)))
    _last_results = res

    # scatter group rows back to segment ids (group sizes vary)
    full = np.zeros((NSEG, D), np.float32)
    for c in range(NCORES):
        oc = np.asarray(res.results[c]["out"], np.float32)
        for gi, (s0, ns) in enumerate(gmaps[c]):
            full[s0 : s0 + ns] = oc[gi * KW : gi * KW + ns]
    return full


# revision 41
# speedup vs baseline: 1.2217x; 1.2200x over previous
"""AttentionPooling (segment softmax pooling) on 8 Trainium2 NeuronCores.

Strategy (data parallel, zero cross-core communication), v6:
  - batch is sorted, so each segment's nodes are contiguous. The host packs
    consecutive segments greedily into groups of <= KW segments and
    <= 128*TPC nodes (one chunk), zero-padding each group to the fixed chunk
    size so the SPMD program has static shapes. Groups are dealt round-robin
    free to cores; every core gets GPC groups (tail cores get empty groups).
  - Host ships xa = fp16(x * a) with a ones-column appended (col D), plus an
    fp8 one-hot segment selector [node -> group-local segment] laid out
    [p, KW, t]. fp16 xa keeps score precision (softmax amplifies score error
    at near-tied segment maxima: bf16 scores land at 1.9e-2 rel err vs the
    2e-2 gate, fp16 at 3.3e-3).
  - Device per 4096-node chunk (= one group):
      tree:  h1 = xa[:,:,0:64] + xa[:,:,64:128]        (DVE fp16 2x)
             h2 = h1[0:32] + h1[32:64]                  (DVE fp16 2x)
             h3 = h2[0:16] + h2[16:32]                  (DVE fp16 2x)
             s  = reduce_x(h3)                          (DVE 1x, 16 wide)
      w = exp(max(s, 0.2 s))        (DVE stt + ACT exp -> bf16)
      onehot fp8 -> bf16            (ACT copy; keeps DVE wscale in 2x mode)
      ohw[p,k,t] = oh[p,k,t]*w[p,t] (DVE tt, w broadcast along k)
      psum[KW, 129] += ohw[:,:,t].T @ xa[:,t,:]  (PE bf16 x fp16, 32 matmuls)
  - Group epilogue: out = psum[:,0:128] * recip(psum[:,128] + 1e-16) * (1/a)
    (pooled values are sums of xa, so dividing by a restores x-pooling);
    DMA to a per-group staging row block; the host scatters group rows back
    to segment ids (group sizes vary, so this mapping is data-dependent).
Padded rows carry xa=0 and an all-zero one-hot row, contributing nothing.
Empty padding groups produce num=0, den=0 -> out 0, discarded by the host.
"""

import numpy as np

N_NODES = 2_000_000
D = 128
NSEG = 16384
NCORES = 8
KW = 32                       # one-hot width: max segments per group
                              # (32 keeps the paired-psum offsets PSUM-legal:
                              # base partitions must be 0/32/64)
TPC = 32                      # tiles per chunk (4096 nodes = one group)
NEG_SLOPE = 0.2
DE = D + 1                    # xa cols: 128 data + ones col

_prog_cache = {}


def _build_program(gpc, tpc=TPC, kw=KW, num_devices=NCORES):
    from concourse import bacc, mybir, tile

    f32 = mybir.dt.float32
    f16 = mybir.dt.float16
    bf16 = mybir.dt.bfloat16
    fp8 = mybir.dt.float8e4

    nc = bacc.Bacc(
        "TRN2",
        target_bir_lowering=False,
        debug=False,
        enable_asserts=False,
        num_devices=num_devices,
    )

    xag = nc.dram_tensor("xag", [gpc, 128, tpc, DE], f16, kind="ExternalInput")
    # onehot ships as fp8 (0/1 exact) to halve its HBM traffic; ACT casts it
    # to bf16 on-chip so the DVE w-scale keeps its 2-byte 2x mode
    ohg = nc.dram_tensor("ohg", [gpc, 128, kw, tpc], fp8, kind="ExternalInput")
    out = nc.dram_tensor("out", [gpc * kw, D], f32, kind="ExternalOutput")

    with tile.TileContext(nc) as tc:
        with (
            tc.tile_pool(name="const", bufs=1) as constp,
            tc.tile_pool(name="xch", bufs=10) as xpool,
            tc.tile_pool(name="oh", bufs=8) as ohpool,
            tc.tile_pool(name="ohb", bufs=6) as ohbpool,
            tc.tile_pool(name="ohw", bufs=6) as ohwpool,
            tc.tile_pool(name="sc", bufs=4) as spool,
            tc.tile_pool(name="ep", bufs=3) as eppool,
            tc.tile_pool(name="ps", bufs=3, space="PSUM") as psump,
        ):
            # Software pipeline, two lag stages: iteration g computes scores
            # for group g (DMA + tree + exp), wscale+matmuls for group g-1,
            # and the epilogue for a finished psum. The lags keep the DVE
            # in-order queue from stalling: wscale(g-1) issues after exp(g-1)
            # already finished, and the epilogue reads a psum whose matmuls
            # finished an iteration ago. Consecutive group PAIRS share one
            # [2*kw]-partition psum tile (disjoint partition ranges) so one
            # epilogue covers two groups.
            prev = None   # (g, xt, oht, wt) awaiting wscale+matmuls
            prev2 = None  # (first_g, psum, nrows) awaiting epilogue
            pair_psum = None

            def mm_stage(g, xt, oht, wt):
                nonlocal pair_psum
                if g % 2 == 0:
                    pair_psum = psump.tile([2 * kw, DE], f32, tag="acc")
                off = (g % 2) * kw
                # w-scaled selector: ohw[p,k,t] = oht[p,k,t] * w[p,t]
                # (w broadcast along k via stride-0 middle dim; last dim
                # stays packed so DVE keeps its 2-byte fast mode)
                ohw = ohwpool.tile([128, kw, tpc], bf16, tag="ohw")
                w_b = wt[:, :].unsqueeze(1).broadcast_to((128, kw, tpc))
                nc.vector.tensor_tensor(
                    ohw[:, :, :], oht[:, :, :], w_b, mybir.AluOpType.mult
                )
                for t in range(tpc):
                    nc.tensor.matmul(
                        pair_psum[off : off + kw, :],
                        ohw[:, :, t],
                        xt[:, t, :],
                        start=(t == 0),
                        stop=(t == tpc - 1),
                    )

            def epi_stage(g0, psum, nrows):
                # den + 1e-16 and num * rden run on ACT (Copy with bias /
                # per-partition scale AP; ACT has PSUM read access), keeping
                # only the reciprocal on DVE. The remaining *(1/a) column
                # correction is applied by the host during the output
                # scatter — the inverse of its xa = x*a preconditioning.
                den = eppool.tile([2 * kw, 1], f32, tag="den")
                nc.scalar.activation(
                    den[0:nrows, :], psum[0:nrows, D : D + 1],
                    mybir.ActivationFunctionType.Copy, bias=1e-16,
                )
                rden = eppool.tile([2 * kw, 1], f32, tag="rden")
                nc.vector.reciprocal(rden[0:nrows, :], den[0:nrows, :])
                osb = eppool.tile([2 * kw, D], f32, tag="osb")
                nc.scalar.activation(
                    osb[0:nrows, :], psum[0:nrows, 0:D],
                    mybir.ActivationFunctionType.Copy,
                    scale=rden[0:nrows, 0:1],
                )
                nc.scalar.dma_start(
                    out=out[g0 * kw : g0 * kw + nrows, :], in_=osb[0:nrows, :]
                )

            for g in range(gpc):
                xt = xpool.tile([128, tpc, DE], f16, tag="x")
                # alternate xa groups between the SP and ACT hardware DMA
                # queues so one queue's DGE bubble overlaps the other's
                xq = nc.sync if g % 2 == 0 else nc.scalar
                xq.dma_start(out=xt[:, :, :], in_=xag[g, :, :, :])
                oh8 = ohpool.tile([128, kw, tpc], fp8, tag="oh")
                # onehot rides the (otherwise idle) gpsimd SWDGE queue
                nc.gpsimd.dma_start(out=oh8[:, :, :], in_=ohg[g, :, :, :])
                oht = ohbpool.tile([128, kw, tpc], bf16, tag="ohb")
                nc.scalar.activation(
                    oht[:, :, :], oh8[:, :, :],
                    mybir.ActivationFunctionType.Copy,
                )

                # scores: tree reduce — three fp16 tensor_tensor add levels
                # run in the DVE 2x 2-byte mode, the final 16-wide
                # tensor_reduce runs 1x (reduce has no 2x uop)
                h1 = spool.tile([128, tpc, D // 2], f16, tag="h1")
                h2 = spool.tile([128, tpc, D // 4], f16, tag="h2")
                h3 = spool.tile([128, tpc, D // 8], f16, tag="h3")
                s16 = spool.tile([128, tpc], f16, tag="s16")
                with nc.allow_low_precision("fp16 score partials"):
                    nc.vector.tensor_tensor(
                        h1[:, :, :], xt[:, :, 0 : D // 2],
                        xt[:, :, D // 2 : D], mybir.AluOpType.add,
                    )
                    nc.vector.tensor_tensor(
                        h2[:, :, :], h1[:, :, 0 : D // 4],
                        h1[:, :, D // 4 : D // 2], mybir.AluOpType.add,
                    )
                    nc.vector.tensor_tensor(
                        h3[:, :, :], h2[:, :, 0 : D // 8],
                        h2[:, :, D // 8 : D // 4], mybir.AluOpType.add,
                    )
                    nc.vector.tensor_reduce(
                        s16[:, :], h3[:, :, :],
                        mybir.AxisListType.X, mybir.AluOpType.add,
                    )
                # leaky relu: max(0.2*s, s) fused on DVE
                l32 = spool.tile([128, tpc], f32, tag="l32")
                nc.vector.scalar_tensor_tensor(
                    l32[:, :], s16[:, :], NEG_SLOPE, s16[:, :],
                    mybir.AluOpType.mult, mybir.AluOpType.max,
                )
                # w = exp(l); no max subtraction needed: scores ~ N(0,11)
                # keep exp(s) inside bf16/fp32 range; ratios unchanged
                wt = spool.tile([128, tpc], bf16, tag="w")
                nc.scalar.activation(
                    wt[:, :], l32[:, :], mybir.ActivationFunctionType.Exp
                )

                if prev2 is not None:
                    epi_stage(*prev2)
                    prev2 = None
                if prev is not None:
                    pg = prev[0]
                    mm_stage(*prev)
                    if pg % 2 == 1:
                        prev2 = (pg - 1, pair_psum, 2 * kw)
                prev = (g, xt, oht, wt)
            if prev2 is not None:
                epi_stage(*prev2)
                prev2 = None
            pg = prev[0]
            mm_stage(*prev)
            if pg % 2 == 1:
                epi_stage(pg - 1, pair_psum, 2 * kw)
            else:
                epi_stage(pg, pair_psum, kw)

    nc.compile()
    return nc


def _prepare_inputs(x, batch, attention_vector):
    """Host-side layout: greedy-pack segments into fixed-size node groups,
    precompute xa = fp16(x*a) and the group-local one-hot selector."""
    x = np.asarray(x, dtype=np.float32)
    batch = np.asarray(batch).astype(np.int64)
    a = np.asarray(attention_vector, dtype=np.float32)
    nseg = NSEG
    cap = 128 * TPC

    counts = np.bincount(batch, minlength=nseg)
    offsets = np.zeros(nseg + 1, np.int64)
    offsets[1:] = np.cumsum(counts)

    # greedy grouping: consecutive segments, <= KW segs and <= cap nodes
    groups = []  # (seg0, nsegs)
    s = 0
    while s < nseg:
        e = s
        nodes = 0
        while e < nseg and e - s < KW and nodes + counts[e] <= cap:
            nodes += counts[e]
            e += 1
        assert e > s, f"segment {s} exceeds group node cap {cap}"
        groups.append((s, e - s))
        s = e
    ngroups = len(groups)
    gpc = (ngroups + NCORES - 1) // NCORES

    xa = (x * a[None, :]).astype(np.float16)
    arecip = (1.0 / a).astype(np.float32)

    from ml_dtypes import float8_e4m3fn

    in_maps = []
    gmaps = []  # per core: list of (seg0, nsegs) per group slot
    for c in range(NCORES):
        gsl = groups[c * gpc : (c + 1) * gpc]
        xag = np.zeros((gpc, cap, DE), np.float16)
        xag[:, :, D] = 1.0
        ohg = np.zeros((gpc, cap, KW), np.float32)
        for gi, (s0, ns) in enumerate(gsl):
            n0, n1 = offsets[s0], offsets[s0 + ns]
            L = n1 - n0
            xag[gi, :L, 0:D] = xa[n0:n1]
            ohg[gi, np.arange(L), batch[n0:n1] - s0] = 1.0
        # [gpc, cap, DE] -> [gpc, 128(p), TPC, DE]
        xag = np.ascontiguousarray(
            xag.reshape(gpc, TPC, 128, DE).transpose(0, 2, 1, 3)
        )
        # [gpc, cap, KW] -> [gpc, 128(p), KW, TPC]
        ohg = np.ascontiguousarray(
            ohg.reshape(gpc, TPC, 128, KW).transpose(0, 2, 3, 1)
        ).astype(float8_e4m3fn)
        in_maps.append({"xag": xag, "ohg": ohg})
        gmaps.append(gsl)
    return in_maps, gmaps, gpc, arecip


_last_results = None


def kernel(x, batch, attention_vector):
    global _last_results
    import os
    from concourse.bass_utils import run_bass_kernel_spmd

    in_maps, gmaps, gpc, arecip = _prepare_inputs(x, batch, attention_vector)
    if gpc not in _prog_cache:
        _prog_cache[gpc] = _build_program(gpc)
    nc = _prog_cache[gpc]
    res = run_bass_kernel_spmd(nc, in_maps, list(range(NCORES)))
    for _ in range(int(os.environ.get("KERNEL_EXTRA_RUNS", "0"))):
        res = run_bass_kernel_spmd(nc, in_maps, list(range(NCORES)))
    _last_results = res

    # scatter group rows back to segment ids (group sizes vary)
    full = np.zeros((NSEG, D), np.float32)
    for c in range(NCORES):
        oc = np.asarray(res.results[c]["out"], np.float32)
        for gi, (s0, ns) in enumerate(gmaps[c]):
            full[s0 : s0 + ns] = oc[gi * KW : gi * KW + ns]
    full *= arecip[None, :]
    return full
